# revision 17
# baseline (speedup 1.0000x reference)
"""MatchingNet model kernel for 8 Trainium2 NeuronCores.

Computation (reference semantics, N=4096, E=512, G=256, V=50000, R=1000):
  x  = embedding[input]          (N, E)
  ex = embedding[set_inputs]     (2, N, E)
  g_out = bidirectional 2-step LSTM over ex   (2, N, E)
  fh = lstm_f(x) + x             (N, E)          [single step, zero state]
  scores[b] = g_out[b] @ fh.T    (2, N, N)
  a = softmax(scores, axis=0)    -- softmax over b (size 2), pointwise in (n,m)
  r[b] = a[b] @ g_out[b]         (2, N, E)
  dot/nr/ng reductions over n -> cos (2, E) -> tiny tail -> softmax (R,)

Sharding: data-parallel over N. Core k owns rows [512k, 512k+512).
v3: LSTMs in bf16 (elementwise chain on gpsimd); attention operands
(fh, dg, g0, g1, a0) in fp8e4m3 (validated on host: ~2e-5 final rel err).
Collectives: AG1 = fh.T fp8 fired right after the f-LSTM; one AG2 = [g1;g0]
rows fp8 after the g cells.  D = (g0-g1) @ fh.T in fp8 DoubleRow;
a0 = sigmoid(D) only (a1 never materialized): the device computes
q = a0 @ g1 and emits sum-stats of q; the host reconstructs the r1 = S - q
statistics algebraically (S = global colsum of g1).  All per-core outputs
are staged in one SBUF tile and written with a single DMA.
"""

import os
import sys

import numpy as np

for _p in ("/opt/trn_rl_repo", os.path.expanduser("~/.axon_site/_ro/trn_rl_repo")):
    if os.path.isdir(_p) and _p not in sys.path:
        sys.path.insert(0, _p)

import ml_dtypes

import concourse.bacc as bacc
import concourse.bass as bass
import concourse.mybir as mybir
import concourse.tile as tile
from concourse import bass_utils
from concourse.masks import make_identity

N, E, G, V, R = 4096, 512, 256, 50000, 1000
NCORES = 8
NL = N // NCORES  # 512 rows per core
P = 128
NE = E // P   # 4 e-chunks
NH = G // P   # 2 hidden chunks for the g-LSTM
NMB = N // P  # 32 m-blocks
EPS = 1e-8

F32 = mybir.dt.float32
BF16 = mybir.dt.bfloat16
FP8 = mybir.dt.float8e4
I32 = mybir.dt.int32
AF = mybir.ActivationFunctionType
ALU = mybir.AluOpType
DR = mybir.MatmulPerfMode.DoubleRow

# staging rows in the (P, 8, NE) output tile
ST_SG0, ST_SG1, ST_SR0, ST_Q2, ST_DOT0, ST_QG, ST_QSUM, ST_CS = range(8)


def _lstm_cell(nc, pools, H, xT, W_sb, U_sb, hprevT, cprevT, bias_sb, h_out,
               c_out, mul_eng):
    """Emit one LSTM cell, transposed layout (feature on partition, n free).

    gates.T[j, n] = sum_e W.T[e, j] x.T[e, n] (+ sum_h U.T[h, j] hprev.T[h, n]) + b[j]
    xT: (P, NE, NL) bf16; W_sb: (P, NE, 4H) bf16; U_sb: (P, H//P, 4H) or None.
    h_out: (P, H//P, NL) bf16; c_out: (P, H//P, NL) f32. When cprevT is None the
    forget gate is skipped (sigmoid(f) * 0) and c = sig(i)*tanh(g).
    mul_eng: engine for the c-chain elementwise ops (h mul stays on vector).
    """
    pg, gp, tp = pools["pg"], pools["gates"], pools["tmp"]
    hc = H // P
    nj = 4 * H // P
    gb = [gp.tile([P, hc, NL], F32, tag=f"gate{g}_h{hc}", bufs=2, name=f"gb{g}")
          for g in range(4)]
    if c_out is None:
        assert cprevT is None
        c_out = gb[1]  # forget-gate buffer is unused for zero-state cells
    for jc in range(nj):
        g = jc // hc
        if cprevT is None and g == 1:
            continue  # forget gate unused with zero initial state
        ps = pg.tile([P, NL], F32, tag="pg", bufs=4, name="ps_gate")
        js = slice(jc * P, (jc + 1) * P)
        for kt in range(NE):
            nc.tensor.matmul(
                ps[:], W_sb[:, kt, js], xT[:, kt, :],
                start=(kt == 0), stop=(U_sb is None and kt == NE - 1))
        if U_sb is not None:
            for kt in range(hc):
                nc.tensor.matmul(
                    ps[:], U_sb[:, kt, js], hprevT[:, kt, :],
                    start=False, stop=(kt == hc - 1))
        func = AF.Tanh if g == 2 else AF.Sigmoid
        nc.scalar.activation(
            out=gb[g][:, jc % hc, :], in_=ps[:], func=func,
            bias=bias_sb[:, jc:jc + 1], scale=1.0)
    for s in range(hc):
        i_, g_, o_ = gb[0][:, s, :], gb[2][:, s, :], gb[3][:, s, :]
        if cprevT is None:
            mul_eng.tensor_mul(c_out[:, s, :], i_, g_)
        else:
            f_ = gb[1][:, s, :]
            ig = tp.tile([P, NL], F32, tag="ig", bufs=2, name="ig")
            mul_eng.tensor_mul(ig[:], i_, g_)
            mul_eng.tensor_mul(c_out[:, s, :], f_, cprevT[:, s, :])
            mul_eng.tensor_add(c_out[:, s, :], c_out[:, s, :], ig[:])
        tc_ = tp.tile([P, NL], F32, tag="tanhc", bufs=2, name="tanhc")
        nc.scalar.activation(out=tc_[:], in_=c_out[:, s, :], func=AF.Tanh)
        nc.vector.tensor_mul(h_out[:, s, :], o_, tc_[:])


def _gather_T(nc, pools, emb, idx_dram, identb, dstT):
    """Gather NL embedding rows, cast bf16, transpose into dstT (P, NE, NL)."""
    ip, rp, pt = pools["idx"], pools["raw"], pools["pt"]
    for t in range(NL // P):
        idx_t = ip.tile([P, 1], I32, tag="idx", bufs=4, name="idx_t")
        nc.sync.dma_start(out=idx_t[:], in_=idx_dram[t * P:(t + 1) * P, :])
        raw = rp.tile([P, E], F32, tag="raw", bufs=4, name="raw")
        nc.gpsimd.indirect_dma_start(
            out=raw[:], out_offset=None, in_=emb[:],
            in_offset=bass.IndirectOffsetOnAxis(ap=idx_t[:, :1], axis=0))
        rawb = rp.tile([P, E], BF16, tag="rawb", bufs=4, name="rawb")
        nc.vector.tensor_copy(out=rawb[:], in_=raw[:])
        for et in range(NE):
            ptile = pt.tile([P, P], BF16, tag="pt", bufs=2, name="ptile")
            nc.tensor.transpose(
                out=ptile[:], in_=rawb[:, et * P:(et + 1) * P], identity=identb[:])
            nc.vector.tensor_copy(
                out=dstT[:, et, t * P:(t + 1) * P], in_=ptile[:])


def build_program():
    nc = bacc.Bacc("TRN2", target_bir_lowering=False, debug=False,
                   enable_asserts=False, num_devices=NCORES)
    dram = lambda name, shape, dt=F32, kind="ExternalInput": \
        nc.dram_tensor(name, shape, dt, kind=kind).ap()

    emb = dram("emb", [V, E])
    idx_x = dram("idx_x", [NL, 1], I32)
    idx_e0 = dram("idx_e0", [NL, 1], I32)
    idx_e1 = dram("idx_e1", [NL, 1], I32)
    # weights pre-laid-out on host as lhsT tiles [p, kt, j] (bf16)
    wgf = dram("wgf", [P, NE, 4 * G], BF16)
    wgr = dram("wgr", [P, NE, 4 * G], BF16)
    ugf = dram("ugf", [P, NH, 4 * G], BF16)
    ugr = dram("ugr", [P, NH, 4 * G], BF16)
    wf = dram("wf", [P, NE, 4 * E], BF16)
    bgf = dram("bgf", [P, 8])
    bgr = dram("bgr", [P, 8])
    bf = dram("bf", [P, 16])
    out_st = dram("out_st", [P, 8 * NE], kind="ExternalOutput")

    with tile.TileContext(nc) as tc:
        _emit(tc, locals())
    nc.compile()
    return nc


def _emit(tc, T):
    nc = tc.nc
    rg = [list(range(NCORES))]
    from contextlib import ExitStack
    ctx = ExitStack()
    with ctx:
        glob = ctx.enter_context(tc.tile_pool(name="glob", bufs=1))
        dramp = ctx.enter_context(tc.tile_pool(name="dramp", bufs=1, space="DRAM"))

        ident = glob.tile([P, P], F32)
        make_identity(nc, ident)
        identb = glob.tile([P, P], BF16)
        nc.vector.tensor_copy(out=identb[:], in_=ident[:])

        # collective bounce buffers, all fp8 (wide-row declarations).
        ag1_src_w = dramp.tile([E // 4, 4 * NL], FP8)                 # fh.T local
        ag1_dst_w = dramp.tile([NCORES * E // 4, 4 * NL], FP8, addr_space="Shared")
        ag1_src = ag1_src_w.rearrange("a (r b) -> (a r) b", r=4)      # (E, NL)
        ag1_dst = ag1_dst_w.rearrange("a (r b) -> (a r) b", r=4)      # (8E, NL)
        ag2a_src_w = dramp.tile([NL // 4, 4 * E], FP8)                # g1 rows
        ag2a_dst_w = dramp.tile([NCORES * NL // 4, 4 * E], FP8,
                                addr_space="Shared")
        ag2a_src = ag2a_src_w.rearrange("a (r b) -> (a r) b", r=4)    # (NL, E)
        ag2a_dst = ag2a_dst_w.rearrange("a (r b) -> (a r) b", r=4)    # (N, E)
        ag2b_src_w = dramp.tile([NL // 4, 4 * E], FP8)                # g0 rows
        ag2b_dst_w = dramp.tile([NCORES * NL // 4, 4 * E], FP8,
                                addr_space="Shared")
        ag2b_src = ag2b_src_w.rearrange("a (r b) -> (a r) b", r=4)
        ag2b_dst = ag2b_dst_w.rearrange("a (r b) -> (a r) b", r=4)

        # long-lived local activations + staging
        g0T = glob.tile([P, NE, NL], BF16)
        g1T = glob.tile([P, NE, NL], BF16)
        dgT8 = glob.tile([P, NE, NL], FP8)
        st = glob.tile([P, 8, NE], F32)

        with tc.tile_pool(name="wpool", bufs=1) as wp, \
             tc.tile_pool(name="acts", bufs=1) as ap_, \
             tc.tile_pool(name="gates", bufs=1) as gp, \
             tc.tile_pool(name="tmp", bufs=1) as tp, \
             tc.tile_pool(name="idx", bufs=1) as ip, \
             tc.tile_pool(name="raw", bufs=1) as rp, \
             tc.tile_pool(name="tps", bufs=1) as tsp, \
             tc.tile_pool(name="pg", bufs=1, space="PSUM") as pgp, \
             tc.tile_pool(name="pt", bufs=1, space="PSUM") as ptp:
            pools = {"pg": pgp, "gates": gp, "tmp": tp, "idx": ip,
                     "raw": rp, "pt": ptp}

            # -- PE warm-up: ~64 back-to-back transposes bring the HAM clock
            # to full rate while the first DMAs are in flight --
            for _ in range(48):
                wtile = ptp.tile([P, P], BF16, tag="pt", bufs=2, name="ptile")
                nc.tensor.transpose(out=wtile[:], in_=identb[:],
                                    identity=identb[:])

            # -- f-LSTM first: x gather + wf load -> fh -> AG1 fires early --
            xT = ap_.tile([P, NE, NL], BF16)
            _gather_T(nc, pools, T["emb"], T["idx_x"], identb, xT)
            wf_sb = wp.tile([P, NE, 4 * E], BF16)
            nc.scalar.dma_start(out=wf_sb[:], in_=T["wf"][:])
            bf_sb = wp.tile([P, 16], F32)
            nc.sync.dma_start(out=bf_sb[:], in_=T["bf"][:])
            w_sb = {}
            for nm, kt in (("wgf", NE), ("wgr", NE)):
                w_sb[nm] = wp.tile([P, kt, 4 * G], BF16, name=nm + "_sb")
                nc.scalar.dma_start(out=w_sb[nm][:], in_=T[nm][:])
            for nm, kt in (("ugf", NH), ("ugr", NH)):
                w_sb[nm] = wp.tile([P, kt, 4 * G], BF16, name=nm + "_sb")
                nc.sync.dma_start(out=w_sb[nm][:], in_=T[nm][:])
            for nm in ("bgf", "bgr"):
                w_sb[nm] = wp.tile([P, 8], F32, name=nm + "_sb")
                nc.sync.dma_start(out=w_sb[nm][:], in_=T[nm][:])

            fh_h = ap_.tile([P, NE, NL], BF16, name="fh_h")
            _lstm_cell(nc, pools, E, xT, wf_sb, None, None, None, bf_sb, fh_h,
                       None, nc.vector)
            fh8 = ap_.tile([P, NE, NL], FP8, name="fh8")
            with tc.high_priority():
                for et in range(NE):
                    nc.vector.tensor_add(fh8[:, et, :], fh_h[:, et, :],
                                         xT[:, et, :])
                    nc.sync.dma_start(
                        out=ag1_src[et * P:(et + 1) * P, :], in_=fh8[:, et, :])
                nc.gpsimd.collective_compute(
                    "AllGather", ALU.bypass, replica_groups=rg,
                    ins=[ag1_src_w[:].opt()], outs=[ag1_dst_w[:].opt()])

            # -- e gathers (DMA overlapped with the f-LSTM above) --
            e0T = ap_.tile([P, NE, NL], BF16)
            e1T = ap_.tile([P, NE, NL], BF16)
            _gather_T(nc, pools, T["emb"], T["idx_e0"], identb, e0T)
            _gather_T(nc, pools, T["emb"], T["idx_e1"], identb, e1T)

            # two independent chains: (fwd0 -> fwd1) and (rev1 -> rev0)
            cfT = ap_.tile([P, NH, NL], F32, name="cfT")
            crT = ap_.tile([P, NH, NL], F32, name="crT")
            c2T = ap_.tile([P, NH, NL], F32, name="c2T")
            c3T = ap_.tile([P, NH, NL], F32, name="c3T")
            hf0 = g0T[:, 0:NH, :]
            hf1 = g1T[:, 0:NH, :]
            hr1 = g1T[:, NH:NE, :]
            hr0 = g0T[:, NH:NE, :]
            _lstm_cell(nc, pools, G, e0T, w_sb["wgf"], None, None, None,
                       w_sb["bgf"], hf0, cfT, nc.vector)
            _lstm_cell(nc, pools, G, e1T, w_sb["wgr"], None, None, None,
                       w_sb["bgr"], hr1, crT, nc.vector)
            _lstm_cell(nc, pools, G, e1T, w_sb["wgf"], w_sb["ugf"], hf0, cfT,
                       w_sb["bgf"], hf1, c2T, nc.vector)
            # g1 = [hf1, hr1] complete: transpose + cast fp8 into ag2 rows 0:NL
            with tc.high_priority():
                for nt in range(NL // P):
                    ptile = ptp.tile([P, E], BF16, tag="ptg", bufs=2,
                                     name="ptg")
                    for et in range(NE):
                        nc.tensor.transpose(
                            out=ptile[:, et * P:(et + 1) * P],
                            in_=g1T[:, et, nt * P:(nt + 1) * P],
                            identity=identb[:])
                    stile = tsp.tile([P, E], FP8, tag="tps", bufs=3,
                                     name="stile")
                    nc.vector.tensor_copy(out=stile[:], in_=ptile[:])
                    nc.sync.dma_start(
                        out=ag2a_src[nt * P:(nt + 1) * P, :], in_=stile[:])
                nc.gpsimd.collective_compute(
                    "AllGather", ALU.bypass, replica_groups=rg,
                    ins=[ag2a_src_w[:].opt()], outs=[ag2a_dst_w[:].opt()])
            for et in range(NH):
                nc.vector.tensor_sub(dgT8[:, et, :], g0T[:, et, :],
                                     g1T[:, et, :])
            _lstm_cell(nc, pools, G, e0T, w_sb["wgr"], w_sb["ugr"], hr1, crT,
                       w_sb["bgr"], hr0, c3T, nc.vector)
            # g0 complete: transpose + cast into ag2 rows NL:2NL, fire AG2
            with tc.high_priority():
                for nt in range(NL // P):
                    ptile = ptp.tile([P, E], BF16, tag="ptg", bufs=2,
                                     name="ptg")
                    for et in range(NE):
                        nc.tensor.transpose(
                            out=ptile[:, et * P:(et + 1) * P],
                            in_=g0T[:, et, nt * P:(nt + 1) * P],
                            identity=identb[:])
                    stile = tsp.tile([P, E], FP8, tag="tps", bufs=3,
                                     name="stile")
                    nc.vector.tensor_copy(out=stile[:], in_=ptile[:])
                    nc.sync.dma_start(
                        out=ag2b_src[nt * P:(nt + 1) * P, :],
                        in_=stile[:])
                nc.gpsimd.collective_compute(
                    "AllGather", ALU.bypass, replica_groups=rg,
                    ins=[ag2b_src_w[:].opt()], outs=[ag2b_dst_w[:].opt()])

            for et in range(NH, NE):
                nc.vector.tensor_sub(dgT8[:, et, :], g0T[:, et, :],
                                     g1T[:, et, :])

        # -- attention phase: bulk preloads + D1 + D2, fp8 DoubleRow --
        attn = ctx.enter_context(tc.tile_pool(name="attn", bufs=1))
        A0T = attn.tile([P, NMB, NL], FP8)
        # bulk preloads of the gathered operands (SBUF-resident for D1/D2)
        fhall = [attn.tile([P, NE, NL], FP8, name=f"fhall{k}")
                 for k in range(NCORES)]
        g1all = [attn.tile([P, 2, 2, E], FP8, name=f"g1all{k}")
                 for k in range(NCORES)]
        g0all = [attn.tile([P, 2, 2, E], FP8, name=f"g0all{k}")
                 for k in range(NCORES)]
        for k in range(NCORES):
            nc.sync.dma_start(
                out=fhall[k][:],
                in_=ag1_dst[k * E:(k + 1) * E, :].rearrange(
                    "(et p) n -> p et n", p=P))
        for k in range(NCORES):
            nc.sync.dma_start(
                out=g1all[k][:],
                in_=ag2a_dst[k * NL:(k + 1) * NL, :].rearrange(
                    "(j j2 p) e -> p j j2 e", p=P, j2=2))
        for k in range(NCORES):
            nc.sync.dma_start(
                out=g0all[k][:],
                in_=ag2b_dst[k * NL:(k + 1) * NL, :].rearrange(
                    "(j j2 p) e -> p j j2 e", p=P, j2=2))

        with tc.tile_pool(name="fin", bufs=1) as fin, \
             tc.tile_pool(name="pd", bufs=1, space="PSUM") as pdp:
            # g/colsum reductions fill the fhall/AG1 wait before D1's matmuls
            for b, gT in ((ST_SG0, g0T), (ST_SG1, g1T)):
                for et in range(NE):
                    scr3 = fin.tile([P, NL], F32, tag="scr3", bufs=2,
                                    name="scr3")
                    nc.scalar.activation(out=scr3[:], in_=gT[:, et, :],
                                         func=AF.Square,
                                         accum_out=st[:, b, et:et + 1])
            for et in range(NE):
                scr4 = fin.tile([P, NL], F32, tag="scr3", bufs=2, name="scr3")
                nc.scalar.activation(out=scr4[:], in_=g1T[:, et, :],
                                     func=AF.Identity,
                                     accum_out=st[:, ST_CS, et:et + 1])

            # phase D1: D.T = fh.T-blocks x dgT; A0 = sigmoid(D) (paired 1024)
            for k in range(NCORES):
                for cp in range(2):
                    pd = pdp.tile([P, 2, NL], F32, tag="pd", bufs=2, name="pd")
                    for half in range(2):
                        c = cp * 2 + half
                        cs = slice(c * P, (c + 1) * P)
                        nc.tensor.matmul(
                            pd[:, half, :], fhall[k][:, 0:2, cs],
                            dgT8[:, 0:2, :], start=True, stop=False,
                            perf_mode=DR)
                        nc.tensor.matmul(
                            pd[:, half, :], fhall[k][:, 2:4, cs],
                            dgT8[:, 2:4, :], start=False, stop=True,
                            perf_mode=DR)
                    mb = k * 4 + cp * 2
                    nc.scalar.activation(
                        out=A0T[:, mb:mb + 2, :], in_=pd[:], func=AF.Sigmoid)

        # ---- phase D2 (et-outer) + phase E per e-chunk ----
        with tc.tile_pool(name="pr", bufs=1, space="PSUM") as prp, \
             tc.tile_pool(name="fin2", bufs=1) as fin:
            r0p = [prp.tile([P, 2, NL], F32, tag=f"r0_{eh}", name=f"r0_{eh}")
                   for eh in range(2)]
            r1p = [prp.tile([P, 2, NL], F32, tag=f"r1_{eh}", name=f"r1_{eh}")
                   for eh in range(2)]
            NPAIR = NMB // 2
            # pass 1: q = a0 @ g1 -- needs only AG2a; j ascending chases the
            # sigmoid production so the tensor queue never stalls after D1
            for j in range(NPAIR):
                k, jj = divmod(j, 2)
                a0 = A0T[:, 2 * j:2 * j + 2, :]
                for et in range(NE):
                    eh, el = divmod(et, 2)
                    es = slice(et * P, (et + 1) * P)
                    nc.tensor.matmul(
                        r1p[eh][:, el, :], g1all[k][:, jj, :, es], a0,
                        start=(j == 0), stop=(j == NPAIR - 1), perf_mode=DR)
            # q stats (scalar/vector run these while the tensor does pass 2)
            for et in range(NE):
                eh, el = divmod(et, 2)
                r1 = r1p[eh][:, el, :]
                scr5 = fin.tile([P, NL], F32, tag="scr5", bufs=2, name="scr5")
                nc.scalar.activation(out=scr5[:], in_=r1, func=AF.Square,
                                     accum_out=st[:, ST_Q2, et:et + 1])
                scr6 = fin.tile([P, NL], F32, tag="scr6", bufs=2, name="scr6")
                nc.scalar.activation(out=scr6[:], in_=r1, func=AF.Identity,
                                     accum_out=st[:, ST_QSUM, et:et + 1])
                scr8 = fin.tile([P, NL], F32, tag="scr8", bufs=2, name="scr8")
                nc.vector.tensor_mul(scr8[:], r1, g1T[:, et, :])
                nc.vector.reduce_sum(out=st[:, ST_QG, et:et + 1],
                                     in_=scr8[:], axis=mybir.AxisListType.X)
            # pass 2: r0 = a0 @ g0 -- needs AG2b
            for j in range(NPAIR):
                k, jj = divmod(j, 2)
                a0 = A0T[:, 2 * j:2 * j + 2, :]
                for et in range(NE):
                    eh, el = divmod(et, 2)
                    es = slice(et * P, (et + 1) * P)
                    nc.tensor.matmul(
                        r0p[eh][:, el, :], g0all[k][:, jj, :, es], a0,
                        start=(j == 0), stop=(j == NPAIR - 1), perf_mode=DR)
            for et in range(NE):
                eh, el = divmod(et, 2)
                r0 = r0p[eh][:, el, :]
                scr2 = fin.tile([P, NL], F32, tag="scr2", bufs=2, name="scr2")
                nc.scalar.activation(out=scr2[:], in_=r0, func=AF.Square,
                                     accum_out=st[:, ST_SR0, et:et + 1])
                scr7 = fin.tile([P, NL], F32, tag="scr7", bufs=2, name="scr7")
                nc.vector.tensor_mul(scr7[:], r0, g0T[:, et, :])
                nc.vector.reduce_sum(out=st[:, ST_DOT0, et:et + 1],
                                     in_=scr7[:], axis=mybir.AxisListType.X)

        # single staged output DMA in natural SBUF layout (host transposes)
        nc.sync.dma_start(
            out=T["out_st"][:].rearrange("p (c et) -> p c et", et=NE),
            in_=st[:])


_PROGRAM = None


def _get_program():
    global _PROGRAM
    if _PROGRAM is None:
        _PROGRAM = build_program()
    return _PROGRAM


def _prep_w(w):
    """(4H, E_in) torch-layout weight -> bf16 lhsT tiles [p, kt, 4H]."""
    wt = np.asarray(w, np.float32).T  # (E_in, 4H)
    e_in, fourh = wt.shape
    t = wt.reshape(e_in // P, P, fourh).transpose(1, 0, 2)
    return np.ascontiguousarray(t.astype(ml_dtypes.bfloat16))


def _prep_b(b1, b2):
    s = (np.asarray(b1, np.float32) + np.asarray(b2, np.float32))
    return np.ascontiguousarray(s.reshape(-1, P).T)


def run_device(inputs, trace=False):
    """Shard inputs, run the 8-core SPMD program, return bass results."""
    nc = _get_program()
    emb = np.ascontiguousarray(np.asarray(inputs["embedding"], np.float32))
    iq = np.asarray(inputs["input"]).astype(np.int32).reshape(N, 1)
    ie = np.asarray(inputs["set_inputs"]).astype(np.int32)
    shared = {
        "emb": emb,
        "wgf": _prep_w(inputs["wih_gf"]), "wgr": _prep_w(inputs["wih_gr"]),
        "ugf": _prep_w(inputs["whh_gf"]), "ugr": _prep_w(inputs["whh_gr"]),
        "wf": _prep_w(inputs["wih_f"]),
        "bgf": _prep_b(inputs["bih_gf"], inputs["bhh_gf"]),
        "bgr": _prep_b(inputs["bih_gr"], inputs["bhh_gr"]),
        "bf": _prep_b(inputs["bih_f"], inputs["bhh_f"]),
    }
    in_maps = []
    for k in range(NCORES):
        sl = slice(k * NL, (k + 1) * NL)
        m = dict(shared)
        m["idx_x"] = np.ascontiguousarray(iq[sl])
        m["idx_e0"] = np.ascontiguousarray(ie[0, sl].reshape(NL, 1))
        m["idx_e1"] = np.ascontiguousarray(ie[1, sl].reshape(NL, 1))
        in_maps.append(m)
    res = bass_utils.run_bass_kernel_spmd(
        nc, in_maps, core_ids=list(range(NCORES)), trace=trace)
    return res


def kernel(**inputs):
    res = run_device(inputs)
    return host_tail(res, inputs)


def host_tail(res, inputs):
    acc = np.zeros((8, E), np.float64)
    for r in res.results:
        a = np.asarray(r["out_st"], np.float64).reshape(P, 8, NE)
        acc += a.transpose(1, 2, 0).reshape(8, E)
    sg0, sg1, sr0, q2, dot0, qg, qsum, cs = acc
    S = cs                                       # global colsum of g1 (E,)
    dot1 = S * S - qg                            # sum_n r1*g1
    sr1 = N * S * S - 2.0 * S * qsum + q2        # sum_n r1^2
    dot = np.stack([dot0, dot1])
    sr = np.stack([sr0, sr1])
    sg = np.stack([sg0, sg1])
    nr = np.maximum(np.sqrt(np.maximum(sr, 0.0)), EPS)
    ng = np.maximum(np.sqrt(np.maximum(sg, 0.0)), EPS)
    cos = dot / (nr * ng)                        # (2, E)
    kern = cos / np.exp(cos).sum()
    w_out = np.asarray(inputs["w_out"], np.float64)
    b_out = np.asarray(inputs["b_out"], np.float64)
    k2 = kern @ w_out.T + b_out                  # (2, R)
    s = k2.sum(axis=1)                           # (2,)
    labels = np.asarray(inputs["set_labels"], np.float64)
    o = s[0] * labels[0] + s[1] * labels[1]      # (R,)
    o = np.exp(o - o.max())
    o /= o.sum()
    return o.astype(np.float32)


# revision 18
# speedup vs baseline: 1.0344x; 1.0344x over previous
"""MatchingNet model kernel for 8 Trainium2 NeuronCores.

Computation (reference semantics, N=4096, E=512, G=256, V=50000, R=1000):
  x  = embedding[input]          (N, E)
  ex = embedding[set_inputs]     (2, N, E)
  g_out = bidirectional 2-step LSTM over ex   (2, N, E)
  fh = lstm_f(x) + x             (N, E)          [single step, zero state]
  scores[b] = g_out[b] @ fh.T    (2, N, N)
  a = softmax(scores, axis=0)    -- softmax over b (size 2), pointwise in (n,m)
  r[b] = a[b] @ g_out[b]         (2, N, E)
  dot/nr/ng reductions over n -> cos (2, E) -> tiny tail -> softmax (R,)

Sharding: data-parallel over N. Core k owns rows [512k, 512k+512).
v3: LSTMs in bf16 (elementwise chain on gpsimd); attention operands
(fh, dg, g0, g1, a0) in fp8e4m3 (validated on host: ~2e-5 final rel err).
Collectives: AG1 = fh.T fp8 fired right after the f-LSTM; one AG2 = [g1;g0]
rows fp8 after the g cells.  D = (g0-g1) @ fh.T in fp8 DoubleRow;
a0 = sigmoid(D) only (a1 never materialized): the device computes
q = a0 @ g1 and emits sum-stats of q; the host reconstructs the r1 = S - q
statistics algebraically (S = global colsum of g1).  All per-core outputs
are staged in one SBUF tile and written with a single DMA.
"""

import os
import sys

import numpy as np

for _p in ("/opt/trn_rl_repo", os.path.expanduser("~/.axon_site/_ro/trn_rl_repo")):
    if os.path.isdir(_p) and _p not in sys.path:
        sys.path.insert(0, _p)

import ml_dtypes

import concourse.bacc as bacc
import concourse.bass as bass
import concourse.mybir as mybir
import concourse.tile as tile
from concourse import bass_utils
from concourse.masks import make_identity

N, E, G, V, R = 4096, 512, 256, 50000, 1000
NCORES = 8
NL = N // NCORES  # 512 rows per core
P = 128
NE = E // P   # 4 e-chunks
NH = G // P   # 2 hidden chunks for the g-LSTM
NMB = N // P  # 32 m-blocks
EPS = 1e-8

F32 = mybir.dt.float32
BF16 = mybir.dt.bfloat16
FP8 = mybir.dt.float8e4
I32 = mybir.dt.int32
AF = mybir.ActivationFunctionType
ALU = mybir.AluOpType
DR = mybir.MatmulPerfMode.DoubleRow

# staging rows in the (P, 8, NE) output tile
ST_SG0, ST_SG1, ST_SR0, ST_Q2, ST_DOT0, ST_QG, ST_QSUM, ST_CS = range(8)


def _lstm_cell(nc, pools, H, xT, W_sb, U_sb, hprevT, cprevT, bias_sb, h_out,
               c_out, mul_eng):
    """Emit one LSTM cell, transposed layout (feature on partition, n free).

    gates.T[j, n] = sum_e W.T[e, j] x.T[e, n] (+ sum_h U.T[h, j] hprev.T[h, n]) + b[j]
    xT: (P, NE, NL) bf16; W_sb: (P, NE, 4H) bf16; U_sb: (P, H//P, 4H) or None.
    h_out: (P, H//P, NL) bf16; c_out: (P, H//P, NL) f32. When cprevT is None the
    forget gate is skipped (sigmoid(f) * 0) and c = sig(i)*tanh(g).
    mul_eng: engine for the c-chain elementwise ops (h mul stays on vector).
    """
    pg, gp, tp = pools["pg"], pools["gates"], pools["tmp"]
    hc = H // P
    nj = 4 * H // P
    gb = [gp.tile([P, hc, NL], F32, tag=f"gate{g}_h{hc}", bufs=2, name=f"gb{g}")
          for g in range(4)]
    if c_out is None:
        assert cprevT is None
        c_out = gb[1]  # forget-gate buffer is unused for zero-state cells
    for jc in range(nj):
        g = jc // hc
        if cprevT is None and g == 1:
            continue  # forget gate unused with zero initial state
        ps = pg.tile([P, NL], F32, tag="pg", bufs=4, name="ps_gate")
        js = slice(jc * P, (jc + 1) * P)
        for kt in range(NE):
            nc.tensor.matmul(
                ps[:], W_sb[:, kt, js], xT[:, kt, :],
                start=(kt == 0), stop=(U_sb is None and kt == NE - 1))
        if U_sb is not None:
            for kt in range(hc):
                nc.tensor.matmul(
                    ps[:], U_sb[:, kt, js], hprevT[:, kt, :],
                    start=False, stop=(kt == hc - 1))
        func = AF.Tanh if g == 2 else AF.Sigmoid
        nc.scalar.activation(
            out=gb[g][:, jc % hc, :], in_=ps[:], func=func,
            bias=bias_sb[:, jc:jc + 1], scale=1.0)
    for s in range(hc):
        i_, g_, o_ = gb[0][:, s, :], gb[2][:, s, :], gb[3][:, s, :]
        if cprevT is None:
            mul_eng.tensor_mul(c_out[:, s, :], i_, g_)
        else:
            f_ = gb[1][:, s, :]
            ig = tp.tile([P, NL], F32, tag="ig", bufs=2, name="ig")
            mul_eng.tensor_mul(ig[:], i_, g_)
            mul_eng.tensor_mul(c_out[:, s, :], f_, cprevT[:, s, :])
            mul_eng.tensor_add(c_out[:, s, :], c_out[:, s, :], ig[:])
        tc_ = tp.tile([P, NL], F32, tag="tanhc", bufs=2, name="tanhc")
        nc.scalar.activation(out=tc_[:], in_=c_out[:, s, :], func=AF.Tanh)
        nc.vector.tensor_mul(h_out[:, s, :], o_, tc_[:])


def _gather_T(nc, pools, emb, idx_dram, identb, dstT):
    """Gather NL embedding rows, cast bf16, transpose into dstT (P, NE, NL)."""
    ip, rp, pt = pools["idx"], pools["raw"], pools["pt"]
    for t in range(NL // P):
        idx_t = ip.tile([P, 1], I32, tag="idx", bufs=4, name="idx_t")
        nc.sync.dma_start(out=idx_t[:], in_=idx_dram[t * P:(t + 1) * P, :])
        raw = rp.tile([P, E], F32, tag="raw", bufs=4, name="raw")
        nc.gpsimd.indirect_dma_start(
            out=raw[:], out_offset=None, in_=emb[:],
            in_offset=bass.IndirectOffsetOnAxis(ap=idx_t[:, :1], axis=0))
        rawb = rp.tile([P, E], BF16, tag="rawb", bufs=4, name="rawb")
        nc.vector.tensor_copy(out=rawb[:], in_=raw[:])
        for et in range(NE):
            ptile = pt.tile([P, P], BF16, tag="pt", bufs=2, name="ptile")
            nc.tensor.transpose(
                out=ptile[:], in_=rawb[:, et * P:(et + 1) * P], identity=identb[:])
            nc.vector.tensor_copy(
                out=dstT[:, et, t * P:(t + 1) * P], in_=ptile[:])


def build_program():
    nc = bacc.Bacc("TRN2", target_bir_lowering=False, debug=False,
                   enable_asserts=False, num_devices=NCORES)
    dram = lambda name, shape, dt=F32, kind="ExternalInput": \
        nc.dram_tensor(name, shape, dt, kind=kind).ap()

    emb = dram("emb", [V, E])
    idx_x = dram("idx_x", [NL, 1], I32)
    idx_e0 = dram("idx_e0", [NL, 1], I32)
    idx_e1 = dram("idx_e1", [NL, 1], I32)
    # weights pre-laid-out on host as lhsT tiles [p, kt, j] (bf16)
    wgf = dram("wgf", [P, NE, 4 * G], BF16)
    wgr = dram("wgr", [P, NE, 4 * G], BF16)
    ugf = dram("ugf", [P, NH, 4 * G], BF16)
    ugr = dram("ugr", [P, NH, 4 * G], BF16)
    wf = dram("wf", [P, NE, 4 * E], BF16)
    bgf = dram("bgf", [P, 8])
    bgr = dram("bgr", [P, 8])
    bf = dram("bf", [P, 16])
    out_st = dram("out_st", [P, 8 * NE], kind="ExternalOutput")

    with tile.TileContext(nc) as tc:
        _emit(tc, locals())
    nc.compile()
    return nc


def _emit(tc, T):
    nc = tc.nc
    rg = [list(range(NCORES))]
    from contextlib import ExitStack
    ctx = ExitStack()
    with ctx:
        glob = ctx.enter_context(tc.tile_pool(name="glob", bufs=1))
        dramp = ctx.enter_context(tc.tile_pool(name="dramp", bufs=1, space="DRAM"))

        ident = glob.tile([P, P], F32)
        make_identity(nc, ident)
        identb = glob.tile([P, P], BF16)
        nc.vector.tensor_copy(out=identb[:], in_=ident[:])

        # collective bounce buffers, all fp8 (wide-row declarations).
        ag1_src_w = dramp.tile([E // 4, 4 * NL], FP8)                 # fh.T local
        ag1_dst_w = dramp.tile([NCORES * E // 4, 4 * NL], FP8, addr_space="Shared")
        ag1_src = ag1_src_w.rearrange("a (r b) -> (a r) b", r=4)      # (E, NL)
        ag1_dst = ag1_dst_w.rearrange("a (r b) -> (a r) b", r=4)      # (8E, NL)
        ag2a_src_w = dramp.tile([NL // 4, 4 * E], FP8)                # g1 rows
        ag2a_dst_w = dramp.tile([NCORES * NL // 4, 4 * E], FP8,
                                addr_space="Shared")
        ag2a_src = ag2a_src_w.rearrange("a (r b) -> (a r) b", r=4)    # (NL, E)
        ag2a_dst = ag2a_dst_w.rearrange("a (r b) -> (a r) b", r=4)    # (N, E)
        ag2b_src_w = dramp.tile([NL // 4, 4 * E], FP8)                # g0 rows
        ag2b_dst_w = dramp.tile([NCORES * NL // 4, 4 * E], FP8,
                                addr_space="Shared")
        ag2b_src = ag2b_src_w.rearrange("a (r b) -> (a r) b", r=4)
        ag2b_dst = ag2b_dst_w.rearrange("a (r b) -> (a r) b", r=4)

        # long-lived local activations + staging
        g0T = glob.tile([P, NE, NL], BF16)
        g1T = glob.tile([P, NE, NL], BF16)
        dgT8 = glob.tile([P, NE, NL], FP8)
        st = glob.tile([P, 8, NE], F32)

        with tc.tile_pool(name="wpool", bufs=1) as wp, \
             tc.tile_pool(name="acts", bufs=1) as ap_, \
             tc.tile_pool(name="gates", bufs=1) as gp, \
             tc.tile_pool(name="tmp", bufs=1) as tp, \
             tc.tile_pool(name="idx", bufs=1) as ip, \
             tc.tile_pool(name="raw", bufs=1) as rp, \
             tc.tile_pool(name="tps", bufs=1) as tsp, \
             tc.tile_pool(name="pg", bufs=1, space="PSUM") as pgp, \
             tc.tile_pool(name="pt", bufs=1, space="PSUM") as ptp:
            pools = {"pg": pgp, "gates": gp, "tmp": tp, "idx": ip,
                     "raw": rp, "pt": ptp}

            # -- PE warm-up: ~64 back-to-back transposes bring the HAM clock
            # to full rate while the first DMAs are in flight --
            for _ in range(48):
                wtile = ptp.tile([P, P], BF16, tag="pt", bufs=2, name="ptile")
                nc.tensor.transpose(out=wtile[:], in_=identb[:],
                                    identity=identb[:])

            # -- f-LSTM first: x gather + wf load -> fh -> AG1 fires early --
            xT = ap_.tile([P, NE, NL], BF16)
            _gather_T(nc, pools, T["emb"], T["idx_x"], identb, xT)
            wf_sb = wp.tile([P, NE, 4 * E], BF16)
            nc.scalar.dma_start(out=wf_sb[:], in_=T["wf"][:])
            bf_sb = wp.tile([P, 16], F32)
            nc.sync.dma_start(out=bf_sb[:], in_=T["bf"][:])
            w_sb = {}
            for nm, kt in (("wgf", NE), ("wgr", NE)):
                w_sb[nm] = wp.tile([P, kt, 4 * G], BF16, name=nm + "_sb")
                nc.scalar.dma_start(out=w_sb[nm][:], in_=T[nm][:])
            for nm, kt in (("ugf", NH), ("ugr", NH)):
                w_sb[nm] = wp.tile([P, kt, 4 * G], BF16, name=nm + "_sb")
                nc.sync.dma_start(out=w_sb[nm][:], in_=T[nm][:])
            for nm in ("bgf", "bgr"):
                w_sb[nm] = wp.tile([P, 8], F32, name=nm + "_sb")
                nc.sync.dma_start(out=w_sb[nm][:], in_=T[nm][:])

            fh_h = ap_.tile([P, NE, NL], BF16, name="fh_h")
            _lstm_cell(nc, pools, E, xT, wf_sb, None, None, None, bf_sb, fh_h,
                       None, nc.vector)
            fh8 = ap_.tile([P, NE, NL], FP8, name="fh8")
            with tc.high_priority():
                for et in range(NE):
                    nc.vector.tensor_add(fh8[:, et, :], fh_h[:, et, :],
                                         xT[:, et, :])
                    nc.sync.dma_start(
                        out=ag1_src[et * P:(et + 1) * P, :], in_=fh8[:, et, :])
                nc.gpsimd.collective_compute(
                    "AllGather", ALU.bypass, replica_groups=rg,
                    ins=[ag1_src_w[:].opt()], outs=[ag1_dst_w[:].opt()])

            # -- e gathers (DMA overlapped with the f-LSTM above) --
            e0T = ap_.tile([P, NE, NL], BF16)
            e1T = ap_.tile([P, NE, NL], BF16)
            _gather_T(nc, pools, T["emb"], T["idx_e0"], identb, e0T)
            _gather_T(nc, pools, T["emb"], T["idx_e1"], identb, e1T)

            # two independent chains: (fwd0 -> fwd1) and (rev1 -> rev0)
            cfT = ap_.tile([P, NH, NL], F32, name="cfT")
            crT = ap_.tile([P, NH, NL], F32, name="crT")
            c2T = ap_.tile([P, NH, NL], F32, name="c2T")
            c3T = ap_.tile([P, NH, NL], F32, name="c3T")
            hf0 = g0T[:, 0:NH, :]
            hf1 = g1T[:, 0:NH, :]
            hr1 = g1T[:, NH:NE, :]
            hr0 = g0T[:, NH:NE, :]
            _lstm_cell(nc, pools, G, e0T, w_sb["wgf"], None, None, None,
                       w_sb["bgf"], hf0, cfT, nc.vector)
            _lstm_cell(nc, pools, G, e1T, w_sb["wgr"], None, None, None,
                       w_sb["bgr"], hr1, crT, nc.vector)
            _lstm_cell(nc, pools, G, e1T, w_sb["wgf"], w_sb["ugf"], hf0, cfT,
                       w_sb["bgf"], hf1, c2T, nc.vector)
            # g1 = [hf1, hr1] complete: transpose + cast fp8 into ag2 rows 0:NL
            with tc.high_priority():
                for nt in range(NL // P):
                    ptile = ptp.tile([P, E], BF16, tag="ptg", bufs=2,
                                     name="ptg")
                    for et in range(NE):
                        nc.tensor.transpose(
                            out=ptile[:, et * P:(et + 1) * P],
                            in_=g1T[:, et, nt * P:(nt + 1) * P],
                            identity=identb[:])
                    stile = tsp.tile([P, E], FP8, tag="tps", bufs=3,
                                     name="stile")
                    nc.vector.tensor_copy(out=stile[:], in_=ptile[:])
                    nc.sync.dma_start(
                        out=ag2a_src[nt * P:(nt + 1) * P, :], in_=stile[:])
                nc.gpsimd.collective_compute(
                    "AllGather", ALU.bypass, replica_groups=rg,
                    ins=[ag2a_src_w[:].opt()], outs=[ag2a_dst_w[:].opt()])
            for et in range(NH):
                nc.vector.tensor_sub(dgT8[:, et, :], g0T[:, et, :],
                                     g1T[:, et, :])
            _lstm_cell(nc, pools, G, e0T, w_sb["wgr"], w_sb["ugr"], hr1, crT,
                       w_sb["bgr"], hr0, c3T, nc.vector)
            # g0 complete: transpose + cast into ag2 rows NL:2NL, fire AG2
            with tc.high_priority():
                for nt in range(NL // P):
                    ptile = ptp.tile([P, E], BF16, tag="ptg", bufs=2,
                                     name="ptg")
                    for et in range(NE):
                        nc.tensor.transpose(
                            out=ptile[:, et * P:(et + 1) * P],
                            in_=g0T[:, et, nt * P:(nt + 1) * P],
                            identity=identb[:])
                    stile = tsp.tile([P, E], FP8, tag="tps", bufs=3,
                                     name="stile")
                    nc.vector.tensor_copy(out=stile[:], in_=ptile[:])
                    nc.sync.dma_start(
                        out=ag2b_src[nt * P:(nt + 1) * P, :],
                        in_=stile[:])
                nc.gpsimd.collective_compute(
                    "AllGather", ALU.bypass, replica_groups=rg,
                    ins=[ag2b_src_w[:].opt()], outs=[ag2b_dst_w[:].opt()])

            for et in range(NH, NE):
                nc.vector.tensor_sub(dgT8[:, et, :], g0T[:, et, :],
                                     g1T[:, et, :])

        # -- attention phase: bulk preloads + D1 + D2, fp8 DoubleRow --
        attn = ctx.enter_context(tc.tile_pool(name="attn", bufs=1))
        A0T = attn.tile([P, NMB, NL], FP8)
        # bulk preloads of the gathered operands (SBUF-resident for D1/D2)
        fhall = [attn.tile([P, NE, NL], FP8, name=f"fhall{k}")
                 for k in range(NCORES)]
        g1all = [attn.tile([P, 2, 2, E], FP8, name=f"g1all{k}")
                 for k in range(NCORES)]
        g0all = [attn.tile([P, 2, 2, E], FP8, name=f"g0all{k}")
                 for k in range(NCORES)]
        for k in range(NCORES):
            nc.sync.dma_start(
                out=fhall[k][:],
                in_=ag1_dst[k * E:(k + 1) * E, :].rearrange(
                    "(et p) n -> p et n", p=P))
        for k in range(NCORES):
            nc.sync.dma_start(
                out=g1all[k][:],
                in_=ag2a_dst[k * NL:(k + 1) * NL, :].rearrange(
                    "(j j2 p) e -> p j j2 e", p=P, j2=2))
        for k in range(NCORES):
            nc.sync.dma_start(
                out=g0all[k][:],
                in_=ag2b_dst[k * NL:(k + 1) * NL, :].rearrange(
                    "(j j2 p) e -> p j j2 e", p=P, j2=2))

        with tc.tile_pool(name="fin", bufs=1) as fin, \
             tc.tile_pool(name="pd", bufs=1, space="PSUM") as pdp:
            # keep the PE clock warm across the AG1-completion wait
            for _ in range(48):
                wu = pdp.tile([P, P], BF16, tag="wu2", bufs=2, name="wu2")
                nc.tensor.transpose(out=wu[:], in_=g0T[:, 0, 0:P],
                                    identity=identb[:])

            # g/colsum reductions fill the fhall/AG1 wait before D1's matmuls
            for b, gT in ((ST_SG0, g0T), (ST_SG1, g1T)):
                for et in range(NE):
                    scr3 = fin.tile([P, NL], F32, tag="scr3", bufs=2,
                                    name="scr3")
                    nc.scalar.activation(out=scr3[:], in_=gT[:, et, :],
                                         func=AF.Square,
                                         accum_out=st[:, b, et:et + 1])
            for et in range(NE):
                scr4 = fin.tile([P, NL], F32, tag="scr3", bufs=2, name="scr3")
                nc.scalar.activation(out=scr4[:], in_=g1T[:, et, :],
                                     func=AF.Identity,
                                     accum_out=st[:, ST_CS, et:et + 1])

            # phase D1: D.T = fh.T-blocks x dgT; A0 = sigmoid(D) (paired 1024)
            for k in range(NCORES):
                for cp in range(2):
                    pd = pdp.tile([P, 2, NL], F32, tag="pd", bufs=2, name="pd")
                    for half in range(2):
                        c = cp * 2 + half
                        cs = slice(c * P, (c + 1) * P)
                        nc.tensor.matmul(
                            pd[:, half, :], fhall[k][:, 0:2, cs],
                            dgT8[:, 0:2, :], start=True, stop=False,
                            perf_mode=DR)
                        nc.tensor.matmul(
                            pd[:, half, :], fhall[k][:, 2:4, cs],
                            dgT8[:, 2:4, :], start=False, stop=True,
                            perf_mode=DR)
                    mb = k * 4 + cp * 2
                    nc.scalar.activation(
                        out=A0T[:, mb:mb + 2, :], in_=pd[:], func=AF.Sigmoid)

        # ---- phase D2 (et-outer) + phase E per e-chunk ----
        with tc.tile_pool(name="pr", bufs=1, space="PSUM") as prp, \
             tc.tile_pool(name="fin2", bufs=1) as fin:
            r0p = [prp.tile([P, 2, NL], F32, tag=f"r0_{eh}", name=f"r0_{eh}")
                   for eh in range(2)]
            r1p = [prp.tile([P, 2, NL], F32, tag=f"r1_{eh}", name=f"r1_{eh}")
                   for eh in range(2)]
            NPAIR = NMB // 2
            # pass 1: q = a0 @ g1 -- needs only AG2a; j ascending chases the
            # sigmoid production so the tensor queue never stalls after D1
            for j in range(NPAIR):
                k, jj = divmod(j, 2)
                a0 = A0T[:, 2 * j:2 * j + 2, :]
                for et in range(NE):
                    eh, el = divmod(et, 2)
                    es = slice(et * P, (et + 1) * P)
                    nc.tensor.matmul(
                        r1p[eh][:, el, :], g1all[k][:, jj, :, es], a0,
                        start=(j == 0), stop=(j == NPAIR - 1), perf_mode=DR)
            # q stats (scalar/vector run these while the tensor does pass 2)
            for et in range(NE):
                eh, el = divmod(et, 2)
                r1 = r1p[eh][:, el, :]
                scr5 = fin.tile([P, NL], F32, tag="scr5", bufs=2, name="scr5")
                nc.scalar.activation(out=scr5[:], in_=r1, func=AF.Square,
                                     accum_out=st[:, ST_Q2, et:et + 1])
                scr6 = fin.tile([P, NL], F32, tag="scr6", bufs=2, name="scr6")
                nc.scalar.activation(out=scr6[:], in_=r1, func=AF.Identity,
                                     accum_out=st[:, ST_QSUM, et:et + 1])
                scr8 = fin.tile([P, NL], F32, tag="scr8", bufs=2, name="scr8")
                nc.vector.tensor_mul(scr8[:], r1, g1T[:, et, :])
                nc.vector.reduce_sum(out=st[:, ST_QG, et:et + 1],
                                     in_=scr8[:], axis=mybir.AxisListType.X)
            # pass 2: r0 = a0 @ g0 -- needs AG2b; et-outer so each e-chunk's
            # stats start while the next chunk's matmuls run
            for et in range(NE):
                eh, el = divmod(et, 2)
                r0 = r0p[eh][:, el, :]
                es = slice(et * P, (et + 1) * P)
                for j in range(NPAIR):
                    k, jj = divmod(j, 2)
                    nc.tensor.matmul(
                        r0, g0all[k][:, jj, :, es], A0T[:, 2 * j:2 * j + 2, :],
                        start=(j == 0), stop=(j == NPAIR - 1), perf_mode=DR)
                scr2 = fin.tile([P, NL], F32, tag="scr2", bufs=2, name="scr2")
                nc.scalar.activation(out=scr2[:], in_=r0, func=AF.Square,
                                     accum_out=st[:, ST_SR0, et:et + 1])
                scr7 = fin.tile([P, NL], F32, tag="scr7", bufs=2, name="scr7")
                nc.vector.tensor_mul(scr7[:], r0, g0T[:, et, :])
                nc.vector.reduce_sum(out=st[:, ST_DOT0, et:et + 1],
                                     in_=scr7[:], axis=mybir.AxisListType.X)

        # single staged output DMA in natural SBUF layout (host transposes)
        nc.sync.dma_start(
            out=T["out_st"][:].rearrange("p (c et) -> p c et", et=NE),
            in_=st[:])


_PROGRAM = None


def _get_program():
    global _PROGRAM
    if _PROGRAM is None:
        _PROGRAM = build_program()
    return _PROGRAM


def _prep_w(w):
    """(4H, E_in) torch-layout weight -> bf16 lhsT tiles [p, kt, 4H]."""
    wt = np.asarray(w, np.float32).T  # (E_in, 4H)
    e_in, fourh = wt.shape
    t = wt.reshape(e_in // P, P, fourh).transpose(1, 0, 2)
    return np.ascontiguousarray(t.astype(ml_dtypes.bfloat16))


def _prep_b(b1, b2):
    s = (np.asarray(b1, np.float32) + np.asarray(b2, np.float32))
    return np.ascontiguousarray(s.reshape(-1, P).T)


def run_device(inputs, trace=False):
    """Shard inputs, run the 8-core SPMD program, return bass results."""
    nc = _get_program()
    emb = np.ascontiguousarray(np.asarray(inputs["embedding"], np.float32))
    iq = np.asarray(inputs["input"]).astype(np.int32).reshape(N, 1)
    ie = np.asarray(inputs["set_inputs"]).astype(np.int32)
    shared = {
        "emb": emb,
        "wgf": _prep_w(inputs["wih_gf"]), "wgr": _prep_w(inputs["wih_gr"]),
        "ugf": _prep_w(inputs["whh_gf"]), "ugr": _prep_w(inputs["whh_gr"]),
        "wf": _prep_w(inputs["wih_f"]),
        "bgf": _prep_b(inputs["bih_gf"], inputs["bhh_gf"]),
        "bgr": _prep_b(inputs["bih_gr"], inputs["bhh_gr"]),
        "bf": _prep_b(inputs["bih_f"], inputs["bhh_f"]),
    }
    in_maps = []
    for k in range(NCORES):
        sl = slice(k * NL, (k + 1) * NL)
        m = dict(shared)
        m["idx_x"] = np.ascontiguousarray(iq[sl])
        m["idx_e0"] = np.ascontiguousarray(ie[0, sl].reshape(NL, 1))
        m["idx_e1"] = np.ascontiguousarray(ie[1, sl].reshape(NL, 1))
        in_maps.append(m)
    res = bass_utils.run_bass_kernel_spmd(
        nc, in_maps, core_ids=list(range(NCORES)), trace=trace)
    return res


def kernel(**inputs):
    res = run_device(inputs)
    return host_tail(res, inputs)


def host_tail(res, inputs):
    acc = np.zeros((8, E), np.float64)
    for r in res.results:
        a = np.asarray(r["out_st"], np.float64).reshape(P, 8, NE)
        acc += a.transpose(1, 2, 0).reshape(8, E)
    sg0, sg1, sr0, q2, dot0, qg, qsum, cs = acc
    S = cs                                       # global colsum of g1 (E,)
    dot1 = S * S - qg                            # sum_n r1*g1
    sr1 = N * S * S - 2.0 * S * qsum + q2        # sum_n r1^2
    dot = np.stack([dot0, dot1])
    sr = np.stack([sr0, sr1])
    sg = np.stack([sg0, sg1])
    nr = np.maximum(np.sqrt(np.maximum(sr, 0.0)), EPS)
    ng = np.maximum(np.sqrt(np.maximum(sg, 0.0)), EPS)
    cos = dot / (nr * ng)                        # (2, E)
    kern = cos / np.exp(cos).sum()
    w_out = np.asarray(inputs["w_out"], np.float64)
    b_out = np.asarray(inputs["b_out"], np.float64)
    k2 = kern @ w_out.T + b_out                  # (2, R)
    s = k2.sum(axis=1)                           # (2,)
    labels = np.asarray(inputs["set_labels"], np.float64)
    o = s[0] * labels[0] + s[1] * labels[1]      # (R,)
    o = np.exp(o - o.max())
    o /= o.sum()
    return o.astype(np.float32)
